# revision 10
# baseline (speedup 1.0000x reference)
"""Trainium2 Bass kernel: 3x3 valid conv (64ch -> 128ch) + per-pixel bias.

Strategy: shard the 510 output rows spatially across 8 NeuronCores (64
rows/core with a 2-row input halo; core 7 overlaps core 6 by 2 rows).
Inside a core, the 64-row band is split across the two PE row-strips:
partitions 0-63 hold the input rows for output rows 0-31 of the band,
partitions 64-127 the rows for output rows 32-63 (the host feeds the
band pre-split so every DMA runs at full 128-partition width).  Each
output row is 9 accumulating K=64 matmuls (one per kernel tap, N=510);
the two strips run concurrently, so a tap-pair costs one N=510 stream.
Bias is added during PSUM evacuation on the Vector engine.

Everything rides in fp16 (x, w, bias, and the output, upcast to fp32
on the host): the PE streams fp16 at the same 1 cycle/row as fp32r,
but HBM/SBUF traffic halves (34.6 MB -> 21.8 MB per core), which takes
the kernel from DMA-bound to compute-bound.  fp16 keeps 10 mantissa
bits, so the absmax rel err stays ~5e-4, well inside the 2e-2 gate.

Schedule: a 128-column slice of the weights loads first and feeds a
short burst of junk matmuls that ramps the PE clock (HAM) out of its
half-rate p-state while the real inputs land.  Weights + per-group
ping-ponged bias tiles ride the sync ring; input chunks ride the
scalar ring.  Both strips' outputs accumulate into two large resident
SBUF tiles, so stores never gate the PE; stores slot into each ring's
idle windows (early groups on scalar behind the input, late groups on
sync behind the bias), and the last group stores row-by-row so the
final drain is a single row per ring.
"""

import numpy as np
from contextlib import ExitStack

import concourse.bass as bass
import concourse.tile as tile
from concourse import bacc, mybir
from concourse import bass_utils

C, H, W = 64, 512, 512
D, KK = 128, 3
OH, OW = H - KK + 1, W - KK + 1          # 510, 510
NCORES = 8
RPC = 64                                  # output rows per core
BAND = RPC + KK - 1                       # 66 input rows per core
HALF = RPC // 2                           # 32 output rows per strip
IBAND = HALF + KK - 1                     # 34 input rows per strip
GROUPS = 8
GROWS = HALF // GROUPS                    # 4 pair-rows per group
NWARM = 30                                # HAM ramp-up junk matmuls

f32 = mybir.dt.float32
f16 = mybir.dt.float16

# row offset of each core's output band
STARTS = [min(i * RPC, OH - RPC) for i in range(NCORES)]

_CACHE = {}

# results of the last hardware run (inspected by test harnesses)
LAST_RESULTS = None


def _build_program():
    nc = bacc.Bacc(
        "TRN2", target_bir_lowering=False, debug=False, num_devices=NCORES
    )
    # x is pre-split on the host: row (h*64+c) holds band rows
    # [32h, 32h+34) of channel c, flattened
    x = nc.dram_tensor("x", [2 * C, IBAND * W], f16, kind="ExternalInput").ap()
    # w is pre-duplicated: rows 0-63 and 64-127 identical, [c, (ky kx d)]
    w = nc.dram_tensor("w", [2 * C, 9 * D], f16, kind="ExternalInput").ap()
    b = nc.dram_tensor("b", [D, RPC, OW], f16, kind="ExternalInput").ap()
    y = nc.dram_tensor("y", [D, RPC, OW], f16, kind="ExternalOutput").ap()

    b_flat = b.rearrange("d r x -> d (r x)")
    y_flat = y.rearrange("d r x -> d (r x)")

    GW = GROWS * OW                       # columns per group

    with tile.TileContext(nc) as tc:
        with ExitStack() as ctx:
            xp = ctx.enter_context(tc.tile_pool(name="xin", bufs=1))
            wp = ctx.enter_context(tc.tile_pool(name="wt", bufs=1))
            bp = ctx.enter_context(tc.tile_pool(name="bias", bufs=1))
            op = ctx.enter_context(tc.tile_pool(name="out", bufs=1))
            pp = ctx.enter_context(tc.tile_pool(name="ps", bufs=3, space="PSUM"))
            wm = ctx.enter_context(tc.tile_pool(name="wm", bufs=1, space="PSUM"))

            # the warm-up burst reads a memset tile, so it needs no DMA
            # round-trip: junk matmuls walk the PE clock (HAM) up to
            # full rate while the real inputs are still in flight
            wz = wp.tile([128, D], f16, tag="wz")
            nc.gpsimd.memset(wz[:], 0.0)
            pw = wm.tile([128, D], f32)
            for _ in range(NWARM):
                nc.tensor.matmul(pw[:], wz[0:64, :], wz[0:64, :])

            # weights in two pieces: taps 0-4 gate the first real
            # matmuls, taps 5-8 arrive while taps 0-4 stream
            wt = wp.tile([128, 9 * D], f16)
            nc.sync.dma_start(wt[:, 0:5 * D], w[:, 0:5 * D])
            nc.sync.dma_start(wt[:, 5 * D:9 * D], w[:, 5 * D:9 * D])

            # input band, both strips; single-row first chunks (tap t
            # of row-pair j only reads band row j + t//3) so compute
            # starts as early as possible, then sizes that stay ahead
            # of the 1-row-per-2.1us consumption rate
            xin = xp.tile([128, IBAND * W], f16)
            bounds = [0, 1, 2, 3, 5, 8, 12, 17, 24, IBAND]
            for ci in range(len(bounds) - 1):
                r0, r1 = bounds[ci], bounds[ci + 1]
                nc.scalar.dma_start(
                    xin[:, r0 * W:r1 * W], x[:, r0 * W:r1 * W]
                )

            # bias: one resident tile per strip, group slices ping-
            # ponged on the sync ring in consumption order
            ba = bp.tile([128, HALF * OW], f16, tag="ba")
            bb = bp.tile([128, HALF * OW], f16, tag="bb")
            for g in range(GROUPS):
                ra, rb = g * GROWS, HALF + g * GROWS
                nc.sync.dma_start(
                    ba[:, g * GW:(g + 1) * GW],
                    b_flat[:, ra * OW:(ra + GROWS) * OW],
                )
                nc.sync.dma_start(
                    bb[:, g * GW:(g + 1) * GW],
                    b_flat[:, rb * OW:(rb + GROWS) * OW],
                )

            # outputs: one resident tile per strip; stores never gate
            # the PE, they just drain into ring idle windows
            ya = op.tile([128, HALF * OW], f16, tag="ya")
            yb = op.tile([128, HALF * OW], f16, tag="yb")

            for g in range(GROUPS):
                ra = g * GROWS                 # band rows ra..ra+3  (strip 0)
                rb = HALF + ra                 # band rows rb..rb+3  (strip 1)

                for j in range(GROWS):
                    yl = ra + j                # strip-local output row
                    pa = pp.tile([128, OW], f32, tag="pa")
                    pb = pp.tile([128, OW], f32, tag="pb")
                    for t in range(9):
                        ky, kx = divmod(t, 3)
                        off = (yl + ky) * W + kx
                        nc.tensor.matmul(
                            pa[:],
                            wt[0:64, t * D:(t + 1) * D],
                            xin[0:64, off:off + OW],
                            start=(t == 0), stop=(t == 8),
                        )
                        nc.tensor.matmul(
                            pb[:],
                            wt[64:128, t * D:(t + 1) * D],
                            xin[64:128, off:off + OW],
                            start=(t == 0), stop=(t == 8),
                        )
                    sl = slice((ra + j) * OW, (ra + j + 1) * OW)
                    if g == GROUPS - 1 and j == GROWS - 1:
                        # final row-pair: strip B evacuates in halves so
                        # its store starts before the full add is done
                        nc.vector.tensor_add(ya[:, sl], pa[:], ba[:, sl])
                        hw_ = OW // 2
                        lo = (ra + j) * OW
                        nc.vector.tensor_add(
                            yb[:, lo:lo + hw_], pb[:, 0:hw_], bb[:, lo:lo + hw_]
                        )
                        nc.vector.tensor_add(
                            yb[:, lo + hw_:lo + OW],
                            pb[:, hw_:OW],
                            bb[:, lo + hw_:lo + OW],
                        )
                    else:
                        nc.vector.tensor_add(ya[:, sl], pa[:], ba[:, sl])
                        nc.vector.tensor_add(yb[:, sl], pb[:], bb[:, sl])

                gsl = slice(ra * OW, (ra + GROWS) * OW)
                if g == GROUPS - 1:
                    # per-row stores, one strip per ring: the final
                    # drain is a single row on each ring
                    for h in range(GROWS - 1):
                        rs = slice((ra + h) * OW, (ra + h + 1) * OW)
                        nc.sync.dma_start(
                            y_flat[:, (ra + h) * OW:(ra + h + 1) * OW],
                            ya[:, rs],
                        )
                        nc.scalar.dma_start(
                            y_flat[:, (rb + h) * OW:(rb + h + 1) * OW],
                            yb[:, rs],
                        )
                    # final row-pair: strip A rides sync whole; strip B
                    # drains as two halves, one per ring, each chasing
                    # its half of the add
                    h = GROWS - 1
                    lo = (ra + h) * OW
                    hw_ = OW // 2
                    nc.sync.dma_start(
                        y_flat[:, lo:lo + OW], ya[:, lo:lo + OW]
                    )
                    nc.scalar.dma_start(
                        y_flat[:, (rb + h) * OW:(rb + h) * OW + hw_],
                        yb[:, lo:lo + hw_],
                    )
                    nc.sync.dma_start(
                        y_flat[:, (rb + h) * OW + hw_:(rb + h + 1) * OW],
                        yb[:, lo + hw_:lo + OW],
                    )
                elif g < 5:
                    # early groups drain behind the input on scalar
                    nc.scalar.dma_start(y_flat[:, gsl], ya[:, gsl])
                    nc.scalar.dma_start(
                        y_flat[:, (rb * OW):(rb + GROWS) * OW], yb[:, gsl]
                    )
                else:
                    # late groups drain behind the bias on sync
                    nc.sync.dma_start(y_flat[:, gsl], ya[:, gsl])
                    nc.sync.dma_start(
                        y_flat[:, (rb * OW):(rb + GROWS) * OW], yb[:, gsl]
                    )

    nc.compile()
    return nc


def kernel(input, kernels, biases):
    global LAST_RESULTS
    if "nc" not in _CACHE:
        _CACHE["nc"] = _build_program()
    nc = _CACHE["nc"]

    xr = np.asarray(input, dtype=np.float32).astype(np.float16)   # [C, H, W]
    w1 = (
        np.ascontiguousarray(np.asarray(kernels, dtype=np.float32).transpose(1, 2, 3, 0))
        .reshape(C, 9 * D)
        .astype(np.float16)
    )
    wr = np.concatenate([w1, w1], axis=0)                         # [128, 9*D]
    br = np.asarray(biases, dtype=np.float32).astype(np.float16)

    in_maps = []
    for s in STARTS:
        band = xr[:, s:s + BAND, :]
        xs = np.concatenate(
            [band[:, 0:IBAND, :], band[:, HALF:HALF + IBAND, :]], axis=0
        ).reshape(2 * C, IBAND * W)
        in_maps.append({
            "x": np.ascontiguousarray(xs),
            "w": wr,
            "b": np.ascontiguousarray(br[:, s:s + RPC, :]),
        })

    res = bass_utils.run_bass_kernel_spmd(
        nc, in_maps, core_ids=list(range(NCORES))
    )
    LAST_RESULTS = res

    out = np.empty((D, OH, OW), np.float32)
    for i, s in enumerate(STARTS):
        out[:, s:s + RPC, :] = res.results[i]["y"].astype(np.float32)
    return out


# revision 11
# speedup vs baseline: 1.0319x; 1.0319x over previous
"""Trainium2 Bass kernel: 3x3 valid conv (64ch -> 128ch) + per-pixel bias.

Strategy: shard the 510 output rows spatially across 8 NeuronCores (64
rows/core with a 2-row input halo; core 7 overlaps core 6 by 2 rows).
Inside a core, the 64-row band is split across the two PE row-strips:
partitions 0-63 hold the input rows for output rows 0-31 of the band,
partitions 64-127 the rows for output rows 32-63 (the host feeds the
band pre-split so every DMA runs at full 128-partition width).  Each
output row is 9 accumulating K=64 matmuls (one per kernel tap, N=510);
the two strips run concurrently, so a tap-pair costs one N=510 stream.
Bias is added during PSUM evacuation on the Vector engine.

Everything rides in fp16 (x, w, bias, and the output, upcast to fp32
on the host): the PE streams fp16 at the same 1 cycle/row as fp32r,
but HBM/SBUF traffic halves (34.6 MB -> 21.8 MB per core), which takes
the kernel from DMA-bound to compute-bound.  fp16 keeps 10 mantissa
bits, so the absmax rel err stays ~5e-4, well inside the 2e-2 gate.

Schedule: a 128-column slice of the weights loads first and feeds a
short burst of junk matmuls that ramps the PE clock (HAM) out of its
half-rate p-state while the real inputs land.  Weights + per-group
ping-ponged bias tiles ride the sync ring; input chunks ride the
scalar ring.  Both strips' outputs accumulate into two large resident
SBUF tiles, so stores never gate the PE; stores slot into each ring's
idle windows (early groups on scalar behind the input, late groups on
sync behind the bias), and the last group stores row-by-row so the
final drain is a single row per ring.
"""

import numpy as np
from contextlib import ExitStack

import concourse.bass as bass
import concourse.tile as tile
from concourse import bacc, mybir
from concourse import bass_utils

C, H, W = 64, 512, 512
D, KK = 128, 3
OH, OW = H - KK + 1, W - KK + 1          # 510, 510
NCORES = 8
RPC = 64                                  # output rows per core
BAND = RPC + KK - 1                       # 66 input rows per core
HALF = RPC // 2                           # 32 output rows per strip
IBAND = HALF + KK - 1                     # 34 input rows per strip
GROUPS = 8
GROWS = HALF // GROUPS                    # 4 pair-rows per group
NWARM = 30                                # HAM ramp-up junk matmuls

f32 = mybir.dt.float32
f16 = mybir.dt.float16

# row offset of each core's output band
STARTS = [min(i * RPC, OH - RPC) for i in range(NCORES)]

_CACHE = {}

# results of the last hardware run (inspected by test harnesses)
LAST_RESULTS = None


def _build_program():
    nc = bacc.Bacc(
        "TRN2", target_bir_lowering=False, debug=False, num_devices=NCORES
    )
    # x is pre-split on the host: row (h*64+c) holds band rows
    # [32h, 32h+34) of channel c, flattened
    x = nc.dram_tensor("x", [2 * C, IBAND * W], f16, kind="ExternalInput").ap()
    # w is pre-duplicated: rows 0-63 and 64-127 identical, [c, (ky kx d)]
    w = nc.dram_tensor("w", [2 * C, 9 * D], f16, kind="ExternalInput").ap()
    b = nc.dram_tensor("b", [D, RPC, OW], f16, kind="ExternalInput").ap()
    y = nc.dram_tensor("y", [D, RPC, OW], f16, kind="ExternalOutput").ap()

    b_flat = b.rearrange("d r x -> d (r x)")
    y_flat = y.rearrange("d r x -> d (r x)")

    GW = GROWS * OW                       # columns per group

    with tile.TileContext(nc) as tc:
        with ExitStack() as ctx:
            xp = ctx.enter_context(tc.tile_pool(name="xin", bufs=1))
            wp = ctx.enter_context(tc.tile_pool(name="wt", bufs=1))
            bp = ctx.enter_context(tc.tile_pool(name="bias", bufs=1))
            op = ctx.enter_context(tc.tile_pool(name="out", bufs=1))
            pp = ctx.enter_context(tc.tile_pool(name="ps", bufs=4, space="PSUM"))

            # the warm-up burst reads a memset tile, so it needs no DMA
            # round-trip: junk matmuls walk the PE clock (HAM) up to
            # full rate while the real inputs are still in flight.  The
            # warm-up PSUM target borrows a slot of the pa ring (its
            # only consumer is the in-order PE itself).
            wz = wp.tile([128, D], f16, tag="wz")
            nc.gpsimd.memset(wz[:], 0.0)
            pw = pp.tile([128, OW], f32, tag="pa")
            for _ in range(NWARM):
                nc.tensor.matmul(pw[:, 0:D], wz[0:64, :], wz[0:64, :])

            wt = wp.tile([128, 9 * D], f16)
            nc.sync.dma_start(wt[:], w[:, :])

            # input band, both strips; single-row first chunks (tap t
            # of row-pair j only reads band row j + t//3) so compute
            # starts as early as possible.  Band row 2 rides the sync
            # ring behind the weights so rows 0-4 land from two rings
            # in parallel, staying ahead of the 1-row-per-1.9us
            # consumption rate through the critical first row-pairs.
            xin = xp.tile([128, IBAND * W], f16)
            nc.scalar.dma_start(xin[:, 0:W], x[:, 0:W])
            nc.scalar.dma_start(xin[:, W:2 * W], x[:, W:2 * W])
            nc.sync.dma_start(xin[:, 2 * W:3 * W], x[:, 2 * W:3 * W])
            bounds = [3, 5, 9, 14, 20, 27, IBAND]
            for ci in range(len(bounds) - 1):
                r0, r1 = bounds[ci], bounds[ci + 1]
                nc.scalar.dma_start(
                    xin[:, r0 * W:r1 * W], x[:, r0 * W:r1 * W]
                )

            # bias: one resident tile per strip, group slices ping-
            # ponged on the sync ring in consumption order
            ba = bp.tile([128, HALF * OW], f16, tag="ba")
            bb = bp.tile([128, HALF * OW], f16, tag="bb")
            for g in range(GROUPS):
                ra, rb = g * GROWS, HALF + g * GROWS
                nc.sync.dma_start(
                    ba[:, g * GW:(g + 1) * GW],
                    b_flat[:, ra * OW:(ra + GROWS) * OW],
                )
                nc.sync.dma_start(
                    bb[:, g * GW:(g + 1) * GW],
                    b_flat[:, rb * OW:(rb + GROWS) * OW],
                )

            # outputs: one resident tile per strip; stores never gate
            # the PE, they just drain into ring idle windows
            ya = op.tile([128, HALF * OW], f16, tag="ya")
            yb = op.tile([128, HALF * OW], f16, tag="yb")

            for g in range(GROUPS):
                ra = g * GROWS                 # band rows ra..ra+3  (strip 0)
                rb = HALF + ra                 # band rows rb..rb+3  (strip 1)

                for j in range(GROWS):
                    yl = ra + j                # strip-local output row
                    pa = pp.tile([128, OW], f32, tag="pa")
                    pb = pp.tile([128, OW], f32, tag="pb")
                    for t in range(9):
                        ky, kx = divmod(t, 3)
                        off = (yl + ky) * W + kx
                        nc.tensor.matmul(
                            pa[:],
                            wt[0:64, t * D:(t + 1) * D],
                            xin[0:64, off:off + OW],
                            start=(t == 0), stop=(t == 8),
                        )
                        nc.tensor.matmul(
                            pb[:],
                            wt[64:128, t * D:(t + 1) * D],
                            xin[64:128, off:off + OW],
                            start=(t == 0), stop=(t == 8),
                        )
                    sl = slice((ra + j) * OW, (ra + j + 1) * OW)
                    if g == GROUPS - 1 and j == GROWS - 1:
                        # final row-pair: strip B evacuates in halves so
                        # its store starts before the full add is done
                        nc.vector.tensor_add(ya[:, sl], pa[:], ba[:, sl])
                        hw_ = OW // 2
                        lo = (ra + j) * OW
                        nc.vector.tensor_add(
                            yb[:, lo:lo + hw_], pb[:, 0:hw_], bb[:, lo:lo + hw_]
                        )
                        nc.vector.tensor_add(
                            yb[:, lo + hw_:lo + OW],
                            pb[:, hw_:OW],
                            bb[:, lo + hw_:lo + OW],
                        )
                    else:
                        nc.vector.tensor_add(ya[:, sl], pa[:], ba[:, sl])
                        nc.vector.tensor_add(yb[:, sl], pb[:], bb[:, sl])

                gsl = slice(ra * OW, (ra + GROWS) * OW)
                if g == GROUPS - 1:
                    # per-row stores, one strip per ring: the final
                    # drain is a single row on each ring
                    for h in range(GROWS - 1):
                        rs = slice((ra + h) * OW, (ra + h + 1) * OW)
                        nc.sync.dma_start(
                            y_flat[:, (ra + h) * OW:(ra + h + 1) * OW],
                            ya[:, rs],
                        )
                        nc.scalar.dma_start(
                            y_flat[:, (rb + h) * OW:(rb + h + 1) * OW],
                            yb[:, rs],
                        )
                    # final row-pair: strip A rides sync whole; strip B
                    # drains as two halves, one per ring, each chasing
                    # its half of the add
                    h = GROWS - 1
                    lo = (ra + h) * OW
                    hw_ = OW // 2
                    nc.sync.dma_start(
                        y_flat[:, lo:lo + OW], ya[:, lo:lo + OW]
                    )
                    nc.scalar.dma_start(
                        y_flat[:, (rb + h) * OW:(rb + h) * OW + hw_],
                        yb[:, lo:lo + hw_],
                    )
                    nc.sync.dma_start(
                        y_flat[:, (rb + h) * OW + hw_:(rb + h + 1) * OW],
                        yb[:, lo + hw_:lo + OW],
                    )
                elif g < 5:
                    # early groups drain behind the input on scalar
                    nc.scalar.dma_start(y_flat[:, gsl], ya[:, gsl])
                    nc.scalar.dma_start(
                        y_flat[:, (rb * OW):(rb + GROWS) * OW], yb[:, gsl]
                    )
                else:
                    # late groups drain behind the bias on sync
                    nc.sync.dma_start(y_flat[:, gsl], ya[:, gsl])
                    nc.sync.dma_start(
                        y_flat[:, (rb * OW):(rb + GROWS) * OW], yb[:, gsl]
                    )

    nc.compile()
    return nc


def kernel(input, kernels, biases):
    global LAST_RESULTS
    if "nc" not in _CACHE:
        _CACHE["nc"] = _build_program()
    nc = _CACHE["nc"]

    xr = np.asarray(input, dtype=np.float32).astype(np.float16)   # [C, H, W]
    w1 = (
        np.ascontiguousarray(np.asarray(kernels, dtype=np.float32).transpose(1, 2, 3, 0))
        .reshape(C, 9 * D)
        .astype(np.float16)
    )
    wr = np.concatenate([w1, w1], axis=0)                         # [128, 9*D]
    br = np.asarray(biases, dtype=np.float32).astype(np.float16)

    in_maps = []
    for s in STARTS:
        band = xr[:, s:s + BAND, :]
        xs = np.concatenate(
            [band[:, 0:IBAND, :], band[:, HALF:HALF + IBAND, :]], axis=0
        ).reshape(2 * C, IBAND * W)
        in_maps.append({
            "x": np.ascontiguousarray(xs),
            "w": wr,
            "b": np.ascontiguousarray(br[:, s:s + RPC, :]),
        })

    res = bass_utils.run_bass_kernel_spmd(
        nc, in_maps, core_ids=list(range(NCORES))
    )
    LAST_RESULTS = res

    out = np.empty((D, OH, OW), np.float32)
    for i, s in enumerate(STARTS):
        out[:, s:s + RPC, :] = res.results[i]["y"].astype(np.float32)
    return out
